# revision 30
# baseline (speedup 1.0000x reference)
"""MoE (top-2 of 8 experts, D=768, FF=3072) on 8 Trainium2 NeuronCores.

Strategy: expert-parallel with capacity factor ~0.97. The router (0.05 GFLOP)
runs on host; tokens are dispatched to their top-2 experts on host, each core
runs one expert's FFN over up to C=992 routed tokens (the 77 GFLOP that
matter), and the host applies the softmax-weighted combine. Tokens routed
beyond an expert's capacity (~3% of pairs for this distribution) are computed
on host, exactly, in fp32 — standard capacity-factor routing except nothing
is dropped. C=992 keeps every core at 2 near-PSUM-max chunks, removing the
pad-to-max-expert imbalance (max count 1065 vs mean 1024) and keeping matmul
instruction count minimal (wide matmuls amortize the per-instr issue residue).

Device layout puts tokens on the matmul free axis, so both matmuls contract
naturally over the partition axis with zero on-device transposes:
    HT[f,t] = relu(sum_d W1[d,f] * XT[d,t] + b1[f])   lhsT=W1, rhs=XT
    YT[d,t] =      sum_f W2[f,d] * HT[f,t] + b2[d]    lhsT=W2, rhs=HT
Inputs are fp16 (well-scaled data; PSUM accumulates fp32), output fp16.
Measured breakdown at 8 cores (exec ~137.6-138.2us): ~12.5us prologue (6us fixed
NEFF init/barriers + crit-bundle DMA, which overlaps the PE clock-ramp
warmup), ~123us matmul stream (PE >99% busy, at the 2.4GHz fp16 roofline),
~5us tail (last output DMA + NEFF teardown/barrier).

Measured dead ends: fp8 DoubleRow matmuls run at 2x fp16 FLOPs (157 TF/s) but
plain-fp8 rel err is 5.3% (gate 2e-2) and the 3-product residual-correction
scheme needs 1.5x the instructions -> slower than fp16. A second HWDGE queue
(scalar engine) splits, not adds, DMA bandwidth (one queue already stripes
across all 16 DMA engines). Splitting the crit bundle delays the w1 stream.
"""

import numpy as np

import concourse.tile as tile
from concourse import bacc, mybir
from concourse import bass_utils

D_MODEL = 768
N_EXPERTS = 8
TOP_K = 2
D_FF = 3072
P = 128
KO = D_MODEL // P     # 6   contraction chunks for MM1 / output tiles for MM2
FO = D_FF // P        # 24  output tiles for MM1 / contraction chunks for MM2
FO_PER_W1 = 3         # w1 streams in slices of 3 f-tiles (after the first tile).
                      # Measured optimum: 2-tile slices (12 DMAs) starve the PE
                      # behind per-DMA issue+ramp overhead (177us vs 149us).
W_PARTS = 4           # w2 DMA split: 4 slices of 6 f-tiles each
FO_PER_PART = FO // W_PARTS
WARMUP_MMS = 12       # dummy matmuls fill the DMA prologue so the PE clock is
                      # fully ramped AND never idles >2us (which drops it back
                      # to 0.8GHz) before the crit bundle lands (~12.3us).
                      # Measured: 7 -> 140.4us median, 10 -> 138.1, 12 -> 137.6,
                      # 14 -> 139.3 (overshoots crit, delays the stream).

_program_cache: dict[tuple, object] = {}


def _token_chunks(C):
    """Equal-ish chunks (multiples of 4, <=512) covering C tokens.

    Equal chunks keep the PE's w1 consumption rate matched to the DMA
    delivery rate from the first matmul on (a smaller first chunk starts
    earlier but outruns the weight stream and stalls, measured slower)."""
    nchunks = -(-C // 512)
    base = -(-C // nchunks)
    base = -(-base // 4) * 4
    chunks = []
    t = 0
    while t < C:
        n = min(base, C - t)
        chunks.append((t, n))
        t += n
    return chunks


def _build_program(C):
    """Bass program for one expert's FFN over C routed tokens (SPMD x8)."""
    key = C
    if key in _program_cache:
        return _program_cache[key]

    fp16 = mybir.dt.float16
    fp32 = mybir.dt.float32
    nc = bacc.Bacc("TRN2", target_bir_lowering=False, debug=False,
                   enable_asserts=True, num_devices=N_EXPERTS)

    chunks = _token_chunks(C)
    cmax = max(n for _, n in chunks)

    # DRAM inputs, pre-sliced host-side so every DMA is contiguous per row.
    # Chunk-0 tokens and w1's FIRST f-tile ride in ONE tensor ("crit"):
    # one DMA issue for exactly what the first matmul group needs; the
    # remaining w1 tiles stream in behind (delivery 0.63us/f-tile beats
    # the PE's consumption). Splitting crit 3-way was measured WORSE: the
    # extra issue slots delay the w1 stream and stall the PE mid-group.
    n0 = chunks[0][1]
    crit_d = nc.dram_tensor("crit", [P, KO, n0 + P], fp16,
                            kind="ExternalInput").ap()
    w1r_d = nc.dram_tensor("w1r", [P, KO, (FO_PER_W1 - 1) * P], fp16,
                           kind="ExternalInput").ap()
    xt_d = [None] + [
        nc.dram_tensor(f"xt{ci}", [P, KO, n], fp16, kind="ExternalInput").ap()
        for ci, (_, n) in list(enumerate(chunks))[1:]]
    w1_d = [None] + [
        nc.dram_tensor(f"w1_{s}", [P, KO, FO_PER_W1 * P], fp16,
                       kind="ExternalInput").ap()
        for s in range(1, FO // FO_PER_W1)]
    w2_d = [nc.dram_tensor(f"w2_{s}", [P, FO_PER_PART, D_MODEL], fp16,
                           kind="ExternalInput").ap() for s in range(W_PARTS)]
    b1_d = nc.dram_tensor("b1c", [P, FO], fp32, kind="ExternalInput").ap()
    b2_d = nc.dram_tensor("b2c", [P, KO], fp32, kind="ExternalInput").ap()
    yt_d = nc.dram_tensor("yt", [P, KO, C], fp16, kind="ExternalOutput").ap()

    with tile.TileContext(nc) as tc:
        with (
            tc.tile_pool(name="wpool", bufs=1) as wpool,
            tc.tile_pool(name="hpool", bufs=2) as hpool,
            tc.tile_pool(name="ypool", bufs=2) as ypool,
            tc.tile_pool(name="pspool", bufs=4, space="PSUM") as pspool,
        ):
            crit_sb = wpool.tile([P, KO, n0 + P], fp16)
            w1r_sb = wpool.tile([P, KO, (FO_PER_W1 - 1) * P], fp16)
            xt_sb = [None] + [
                wpool.tile([P, KO, n], fp16, name=f"xt_sb{ci}")
                for ci, (_, n) in list(enumerate(chunks))[1:]]
            w1_sb = [
                wpool.tile([P, KO, FO_PER_W1 * P], fp16, name=f"w1_sb{s}")
                for s in range(1, FO // FO_PER_W1)]

            def xt_ap(ci, ko, nt):
                """rhs AP for token chunk `ci`, contraction tile `ko`."""
                if ci == 0:
                    return crit_sb[:, ko, :nt]
                return xt_sb[ci][:, ko, :nt]

            def w1_ap(fo, ko):
                """lhsT AP for w1 f-tile `fo`, contraction tile `ko`."""
                if fo == 0:
                    return crit_sb[:, ko, n0:]
                if fo < FO_PER_W1:
                    return w1r_sb[:, ko, (fo - 1) * P:fo * P]
                t = w1_sb[fo // FO_PER_W1 - 1]
                f = fo % FO_PER_W1
                return t[:, ko, f * P:(f + 1) * P]
            w2_sb = [wpool.tile([P, FO_PER_PART, D_MODEL], fp16, name=f"w2_sb{s}")
                     for s in range(W_PARTS)]
            b1_sb = wpool.tile([P, FO], fp32)
            b2_sb = wpool.tile([P, KO], fp32)

            # PE warmup: dummy matmuls on a zeroed tile fill the DMA
            # prologue so the HAM clock-gate reaches 2.4GHz before the
            # real matmuls start.
            warm = wpool.tile([P, 512], fp16)
            nc.vector.memset(warm[:], 0.0)
            ps_w = pspool.tile([P, 512], fp32, name="ps_w", bufs=1)
            for _ in range(WARMUP_MMS):
                nc.tensor.matmul(ps_w[:], lhsT=warm[:, :P], rhs=warm[:],
                                 start=True, stop=True)

            # DMA order = need order, all on the sync HWDGE queue (a single
            # queue already stripes across all 16 DMA engines and saturates
            # the per-core HBM path; a second queue just steals bandwidth
            # from the w1 stream - measured 14us WORSE).
            nc.sync.dma_start(crit_sb[:], crit_d[:])
            nc.sync.dma_start(w1r_sb[:], w1r_d[:])
            for s in range(1, FO // FO_PER_W1):
                nc.sync.dma_start(w1_sb[s - 1][:], w1_d[s][:])
                if s == 1:
                    # b1 (12KB) is not needed until the first epilogue;
                    # issuing it here keeps w1r/w1_1's issue slots early
                    nc.sync.dma_start(b1_sb[:], b1_d[:])
            for ci in range(1, len(chunks)):
                nc.sync.dma_start(xt_sb[ci][:], xt_d[ci][:])
            for s in range(W_PARTS):
                nc.sync.dma_start(w2_sb[s][:], w2_d[s][:])
            nc.sync.dma_start(b2_sb[:], b2_d[:])

            for ci, (t0, nt) in enumerate(chunks):
                ht = hpool.tile([P, FO, cmax], fp16, name="ht")
                for fo in range(FO):
                    ps = pspool.tile([P, cmax], fp32, name="ps")
                    for ko in range(KO):
                        nc.tensor.matmul(
                            ps[:, :nt],
                            lhsT=w1_ap(fo, ko),
                            rhs=xt_ap(ci, ko, nt),
                            start=(ko == 0), stop=(ko == KO - 1),
                        )
                    nc.scalar.activation(
                        ht[:, fo, :nt], ps[:, :nt],
                        mybir.ActivationFunctionType.Relu,
                        bias=b1_sb[:, fo:fo + 1],
                    )
                yt = ypool.tile([P, KO, cmax], fp16, name="yt")
                last_chunk = ci == len(chunks) - 1
                for ko in range(KO):
                    # The very last group is split column-wise in half so
                    # the first half's epilogue+DMA overlap the second
                    # half's matmuls, shortening the critical tail.
                    if last_chunk and ko == KO - 1:
                        nh = ((nt // 2) + 3) // 4 * 4
                        cols = [(0, nh), (nh, nt - nh)]
                    else:
                        cols = [(0, nt)]
                    for c0, cn in cols:
                        ps = pspool.tile([P, cmax], fp32, name="ps")
                        for fo in range(FO):
                            s, f = divmod(fo, FO_PER_PART)
                            nc.tensor.matmul(
                                ps[:, :cn],
                                lhsT=w2_sb[s][:, f, ko * P:(ko + 1) * P],
                                rhs=ht[:, fo, c0:c0 + cn],
                                start=(fo == 0), stop=(fo == FO - 1),
                            )
                        # DVE is ~3x faster than ACT for the plain bias-add
                        # drain; the final one is on the critical tail.
                        nc.vector.tensor_scalar_add(
                            yt[:, ko, c0:c0 + cn], ps[:, :cn],
                            b2_sb[:, ko:ko + 1])
                        nc.sync.dma_start(yt_d[:, ko, t0 + c0:t0 + c0 + cn],
                                          yt[:, ko, c0:c0 + cn])

    nc.compile()
    _program_cache[key] = nc
    return nc


def _route(xf, Wr):
    """Host router: top-2 expert ids + softmax weights (matches lax.top_k)."""
    T = xf.shape[0]
    logits = xf @ Wr
    i1 = np.argmax(logits, axis=1)
    l1 = logits[np.arange(T), i1]
    masked = logits.copy()
    masked[np.arange(T), i1] = -np.inf
    i2 = np.argmax(masked, axis=1)
    l2 = logits[np.arange(T), i2]
    e2 = np.exp((l2 - l1).astype(np.float32))
    wt1 = 1.0 / (1.0 + e2)
    wt2 = e2 / (1.0 + e2)
    return i1, i2, wt1, wt2


def _forward(inputs, trace=False, trace_kwargs=None):
    x = np.ascontiguousarray(np.asarray(inputs["x"], dtype=np.float32))
    Wr = np.asarray(inputs["Wr"], dtype=np.float32)
    W1 = np.asarray(inputs["W1"], dtype=np.float32)
    b1 = np.asarray(inputs["b1"], dtype=np.float32)
    W2 = np.asarray(inputs["W2"], dtype=np.float32)
    b2 = np.asarray(inputs["b2"], dtype=np.float32)

    B, S, D = x.shape
    T = B * S
    xf = x.reshape(T, D)

    i1, i2, wt1, wt2 = _route(xf, Wr)
    idx = [np.nonzero((i1 == e) | (i2 == e))[0] for e in range(N_EXPERTS)]
    gw = [np.where(i1[ix] == e, wt1[ix], wt2[ix]).astype(np.float32)
          for e, ix in enumerate(idx)]

    # Capacity factor ~0.97: each core takes at most C=992 tokens; overflow
    # pairs (~3% for this distribution) are computed on host in fp32.
    C = 992
    overflow = [(e, idx[e][C:], gw[e][C:]) for e in range(N_EXPERTS)
                if len(idx[e]) > C]
    idx = [ix[:C] for ix in idx]
    gw = [w[:C] for w in gw]

    nc = _build_program(C)
    chunks = _token_chunks(C)

    in_maps = []
    for e in range(N_EXPERTS):
        ix = idx[e]
        xe = np.zeros((C, D), dtype=np.float16)
        xe[:len(ix)] = xf[ix]
        # XT[d,t] -> [p, ko, t] with d = ko*P + p
        xt = np.ascontiguousarray(xe.T.reshape(KO, P, C).transpose(1, 0, 2))
        w1 = np.ascontiguousarray(
            W1[e].astype(np.float16).reshape(KO, P, D_FF).transpose(1, 0, 2))
        w2 = np.ascontiguousarray(
            W2[e].astype(np.float16).reshape(FO, P, D_MODEL).transpose(1, 0, 2))
        m = {"b1c": np.ascontiguousarray(b1[e].reshape(FO, P).T),
             "b2c": np.ascontiguousarray(b2[e].reshape(KO, P).T)}
        n0 = chunks[0][1]
        m["crit"] = np.ascontiguousarray(
            np.concatenate([xt[:, :, :n0], w1[:, :, :P]], axis=2))
        m["w1r"] = np.ascontiguousarray(w1[:, :, P:FO_PER_W1 * P])
        for ci, (t0, n) in list(enumerate(chunks))[1:]:
            m[f"xt{ci}"] = np.ascontiguousarray(xt[:, :, t0:t0 + n])
        for s in range(1, FO // FO_PER_W1):
            f0 = s * FO_PER_W1 * P
            m[f"w1_{s}"] = np.ascontiguousarray(w1[:, :, f0:f0 + FO_PER_W1 * P])
        for s in range(W_PARTS):
            m[f"w2_{s}"] = np.ascontiguousarray(
                w2[:, s * FO_PER_PART:(s + 1) * FO_PER_PART, :])
        in_maps.append(m)

    try:
        res = bass_utils.run_bass_kernel_spmd(
            nc, in_maps, core_ids=list(range(N_EXPERTS)), trace=trace,
            **(trace_kwargs or {}),
        )
    except Exception:
        # transient device errors (NRT_EXEC_UNIT_UNRECOVERABLE) have been
        # observed once under rapid successive loads; one retry clears them
        res = bass_utils.run_bass_kernel_spmd(
            nc, in_maps, core_ids=list(range(N_EXPERTS)), trace=trace,
            **(trace_kwargs or {}),
        )

    out = np.zeros((T, D), dtype=np.float32)
    for e in range(N_EXPERTS):
        ix = idx[e]
        if len(ix) == 0:
            continue
        # yt [p, ko, t] -> Y [t, d]
        yt = res.results[e]["yt"].astype(np.float32)
        ye = yt.transpose(2, 1, 0).reshape(C, D)[:len(ix)]
        out[ix] += gw[e][:, None] * ye
    for e, ix, w in overflow:
        h = np.maximum(xf[ix] @ W1[e] + b1[e], 0.0)
        out[ix] += w[:, None] * (h @ W2[e] + b2[e])
    return out.reshape(B, S, D), res


def kernel(**inputs) -> np.ndarray:
    out, _ = _forward(inputs)
    return out



# revision 32
# speedup vs baseline: 1.0268x; 1.0268x over previous
"""MoE (top-2 of 8 experts, D=768, FF=3072) on 8 Trainium2 NeuronCores.

Strategy: expert-parallel with capacity factor ~0.97. The router (0.05 GFLOP)
runs on host; tokens are dispatched to their top-2 experts on host, each core
runs one expert's FFN over up to C=992 routed tokens (the 77 GFLOP that
matter), and the host applies the softmax-weighted combine. Tokens routed
beyond an expert's capacity (~3% of pairs for this distribution) are computed
on host, exactly, in fp32 — standard capacity-factor routing except nothing
is dropped. C=992 keeps every core at 2 near-PSUM-max chunks, removing the
pad-to-max-expert imbalance (max count 1065 vs mean 1024) and keeping matmul
instruction count minimal (wide matmuls amortize the per-instr issue residue).

Device layout puts tokens on the matmul free axis, so both matmuls contract
naturally over the partition axis with zero on-device transposes:
    HT[f,t] = relu(sum_d W1[d,f] * XT[d,t] + b1[f])   lhsT=W1, rhs=XT
    YT[d,t] =      sum_f W2[f,d] * HT[f,t] + b2[d]    lhsT=W2, rhs=HT
Inputs are fp16 (well-scaled data; PSUM accumulates fp32), output fp16.

Mixed-precision MM2: the LAST 256 of MM2's 3072-deep contraction run as ONE
fp8e4m3 DoubleRow matmul per accumulation group (contracts 2 k-tiles per
instruction at fp16-instruction cost), replacing two fp16 matmuls — ~2.1us
off the stream. Quantizing a 1/12 fraction of the contraction scales the
full-fp8 error (5.3%) by sqrt(1/12): measured rel err 1.07e-2 vs the 2e-2
gate. Scales are chosen so no epilogue changes are needed: h8 = fp8(h/8)
(written directly by the ACT relu with scale=1/8; b1c cols 22-23 pre-scaled)
and w28 = fp8(8*W2[2816:]), so h8@w28 lands at exactly the fp16 partials'
scale and accumulates into the same PSUM group. Both operands sit in the
e4m3 normal range (raw W2 ~0.02 would be denormal, rel err ~10-20%).

Measured breakdown at 8 cores (exec ~135.4-138.5us): ~12.5us prologue (6us
fixed NEFF init/barriers + crit-bundle DMA, which overlaps the PE clock-ramp
warmup), ~120.5us matmul stream (PE >99% busy at the 2.4GHz roofline),
~5us tail (last output DMA + NEFF teardown/barrier).

Measured dead ends: FULL-fp8 DoubleRow (2x FLOPs) has 5.3% rel err and the
3-product residual-correction scheme needs 1.5x the instructions -> slower
than fp16; going to a 2/12 fp8 fraction (~1.5% err) leaves too little gate
margin. A second HWDGE queue (scalar engine) splits, not adds, DMA bandwidth
(one queue already stripes across all 16 DMA engines). Splitting the crit
bundle delays the w1 stream. Final-group column splits below ~229 cols go
LDWEIGHTS-bound and add PE time (248/248 is the sweet spot).
"""

import ml_dtypes
import numpy as np

import concourse.tile as tile
from concourse import bacc, mybir
from concourse import bass_utils

D_MODEL = 768
N_EXPERTS = 8
TOP_K = 2
D_FF = 3072
P = 128
KO = D_MODEL // P     # 6   contraction chunks for MM1 / output tiles for MM2
FO = D_FF // P        # 24  output tiles for MM1 / contraction chunks for MM2
FO_PER_W1 = 3         # w1 streams in slices of 3 f-tiles (after the first tile).
                      # Measured optimum: 2-tile slices (12 DMAs) starve the PE
                      # behind per-DMA issue+ramp overhead (177us vs 149us).
W_PARTS = 4           # w2 DMA split: 4 slices of 6 f-tiles each
FO_PER_PART = FO // W_PARTS
WARMUP_MMS = 12       # dummy matmuls fill the DMA prologue so the PE clock is
                      # fully ramped AND never idles >2us (which drops it back
                      # to 0.8GHz) before the crit bundle lands (~12.3us).
                      # Measured: 7 -> 140.4us median, 10 -> 138.1, 12 -> 137.6,
                      # 14 -> 139.3 (overshoots crit, delays the stream).

_program_cache: dict[tuple, object] = {}


def _token_chunks(C):
    """Equal-ish chunks (multiples of 4, <=512) covering C tokens.

    Equal chunks keep the PE's w1 consumption rate matched to the DMA
    delivery rate from the first matmul on (a smaller first chunk starts
    earlier but outruns the weight stream and stalls, measured slower)."""
    nchunks = -(-C // 512)
    base = -(-C // nchunks)
    base = -(-base // 4) * 4
    chunks = []
    t = 0
    while t < C:
        n = min(base, C - t)
        chunks.append((t, n))
        t += n
    return chunks


def _build_program(C):
    """Bass program for one expert's FFN over C routed tokens (SPMD x8)."""
    key = C
    if key in _program_cache:
        return _program_cache[key]

    fp16 = mybir.dt.float16
    fp32 = mybir.dt.float32
    fp8 = mybir.dt.float8e4
    nc = bacc.Bacc("TRN2", target_bir_lowering=False, debug=False,
                   enable_asserts=True, num_devices=N_EXPERTS)

    chunks = _token_chunks(C)
    cmax = max(n for _, n in chunks)

    # DRAM inputs, pre-sliced host-side so every DMA is contiguous per row.
    # Chunk-0 tokens and w1's FIRST f-tile ride in ONE tensor ("crit"):
    # one DMA issue for exactly what the first matmul group needs; the
    # remaining w1 tiles stream in behind (delivery 0.63us/f-tile beats
    # the PE's consumption). Splitting crit 3-way was measured WORSE: the
    # extra issue slots delay the w1 stream and stall the PE mid-group.
    n0 = chunks[0][1]
    crit_d = nc.dram_tensor("crit", [P, KO, n0 + P], fp16,
                            kind="ExternalInput").ap()
    w1r_d = nc.dram_tensor("w1r", [P, KO, (FO_PER_W1 - 1) * P], fp16,
                           kind="ExternalInput").ap()
    xt_d = [None] + [
        nc.dram_tensor(f"xt{ci}", [P, KO, n], fp16, kind="ExternalInput").ap()
        for ci, (_, n) in list(enumerate(chunks))[1:]]
    w1_d = [None] + [
        nc.dram_tensor(f"w1_{s}", [P, KO, FO_PER_W1 * P], fp16,
                       kind="ExternalInput").ap()
        for s in range(1, FO // FO_PER_W1)]
    w2_d = [nc.dram_tensor(f"w2_{s}", [P, FO_PER_PART, D_MODEL], fp16,
                           kind="ExternalInput").ap() for s in range(W_PARTS)]
    w28_d = nc.dram_tensor("w28", [P, 2, KO * P], fp8,
                           kind="ExternalInput").ap()
    b1_d = nc.dram_tensor("b1c", [P, FO], fp32, kind="ExternalInput").ap()
    b2_d = nc.dram_tensor("b2c", [P, KO], fp32, kind="ExternalInput").ap()
    yt_d = nc.dram_tensor("yt", [P, KO, C], fp16, kind="ExternalOutput").ap()

    with tile.TileContext(nc) as tc:
        with (
            tc.tile_pool(name="wpool", bufs=1) as wpool,
            tc.tile_pool(name="hpool", bufs=2) as hpool,
            tc.tile_pool(name="ypool", bufs=2) as ypool,
            tc.tile_pool(name="pspool", bufs=4, space="PSUM") as pspool,
        ):
            crit_sb = wpool.tile([P, KO, n0 + P], fp16)
            w1r_sb = wpool.tile([P, KO, (FO_PER_W1 - 1) * P], fp16)
            xt_sb = [None] + [
                wpool.tile([P, KO, n], fp16, name=f"xt_sb{ci}")
                for ci, (_, n) in list(enumerate(chunks))[1:]]
            w1_sb = [
                wpool.tile([P, KO, FO_PER_W1 * P], fp16, name=f"w1_sb{s}")
                for s in range(1, FO // FO_PER_W1)]

            def xt_ap(ci, ko, nt):
                """rhs AP for token chunk `ci`, contraction tile `ko`."""
                if ci == 0:
                    return crit_sb[:, ko, :nt]
                return xt_sb[ci][:, ko, :nt]

            def w1_ap(fo, ko):
                """lhsT AP for w1 f-tile `fo`, contraction tile `ko`."""
                if fo == 0:
                    return crit_sb[:, ko, n0:]
                if fo < FO_PER_W1:
                    return w1r_sb[:, ko, (fo - 1) * P:fo * P]
                t = w1_sb[fo // FO_PER_W1 - 1]
                f = fo % FO_PER_W1
                return t[:, ko, f * P:(f + 1) * P]
            w2_sb = [wpool.tile([P, FO_PER_PART, D_MODEL], fp16, name=f"w2_sb{s}")
                     for s in range(W_PARTS)]
            w28_sb = wpool.tile([P, 2, KO * P], fp8)
            b1_sb = wpool.tile([P, FO], fp32)
            b2_sb = wpool.tile([P, KO], fp32)

            # PE warmup: dummy matmuls on a zeroed tile fill the DMA
            # prologue so the HAM clock-gate reaches 2.4GHz before the
            # real matmuls start.
            warm = wpool.tile([P, 512], fp16)
            nc.vector.memset(warm[:], 0.0)
            ps_w = pspool.tile([P, 512], fp32, name="ps_w", bufs=1)
            for _ in range(WARMUP_MMS):
                nc.tensor.matmul(ps_w[:], lhsT=warm[:, :P], rhs=warm[:],
                                 start=True, stop=True)

            # DMA order = need order, all on the sync HWDGE queue (a single
            # queue already stripes across all 16 DMA engines and saturates
            # the per-core HBM path; a second queue just steals bandwidth
            # from the w1 stream - measured 14us WORSE).
            nc.sync.dma_start(crit_sb[:], crit_d[:])
            nc.sync.dma_start(w1r_sb[:], w1r_d[:])
            for s in range(1, FO // FO_PER_W1):
                nc.sync.dma_start(w1_sb[s - 1][:], w1_d[s][:])
                if s == 1:
                    # b1 (12KB) is not needed until the first epilogue;
                    # issuing it here keeps w1r/w1_1's issue slots early
                    nc.sync.dma_start(b1_sb[:], b1_d[:])
            for ci in range(1, len(chunks)):
                nc.sync.dma_start(xt_sb[ci][:], xt_d[ci][:])
            for s in range(W_PARTS):
                nc.sync.dma_start(w2_sb[s][:], w2_d[s][:])
            nc.sync.dma_start(w28_sb[:], w28_d[:])
            nc.sync.dma_start(b2_sb[:], b2_d[:])

            for ci, (t0, nt) in enumerate(chunks):
                ht = hpool.tile([P, FO, cmax], fp16, name="ht")
                ht8 = hpool.tile([P, 2, cmax], fp8, name="ht8")
                for fo in range(FO):
                    ps = pspool.tile([P, cmax], fp32, name="ps")
                    for ko in range(KO):
                        nc.tensor.matmul(
                            ps[:, :nt],
                            lhsT=w1_ap(fo, ko),
                            rhs=xt_ap(ci, ko, nt),
                            start=(ko == 0), stop=(ko == KO - 1),
                        )
                    if fo >= FO - 2:
                        # Mixed-precision MM2 (see header): the last two
                        # f-tiles of h are written as fp8 e4m3 at 1/8 scale
                        # (values land in the e4m3 normal range; b1c cols
                        # 22-23 are pre-scaled by 1/8 host-side) so one
                        # DoubleRow matmul against 8*W2 replaces two fp16
                        # matmuls per MM2 group at identical PSUM scale.
                        nc.scalar.activation(
                            ht8[:, fo - (FO - 2), :nt], ps[:, :nt],
                            mybir.ActivationFunctionType.Relu,
                            bias=b1_sb[:, fo:fo + 1], scale=0.125,
                        )
                    else:
                        nc.scalar.activation(
                            ht[:, fo, :nt], ps[:, :nt],
                            mybir.ActivationFunctionType.Relu,
                            bias=b1_sb[:, fo:fo + 1],
                        )
                yt = ypool.tile([P, KO, cmax], fp16, name="yt")
                last_chunk = ci == len(chunks) - 1
                for ko in range(KO):
                    # The very last group is split column-wise in half so
                    # the first half's epilogue+DMA overlap the second
                    # half's matmuls, shortening the critical tail.
                    if last_chunk and ko == KO - 1:
                        nh = ((nt // 2) + 3) // 4 * 4
                        cols = [(0, nh), (nh, nt - nh)]
                    else:
                        cols = [(0, nt)]
                    for c0, cn in cols:
                        ps = pspool.tile([P, cmax], fp32, name="ps")
                        for fo in range(FO - 2):
                            s, f = divmod(fo, FO_PER_PART)
                            nc.tensor.matmul(
                                ps[:, :cn],
                                lhsT=w2_sb[s][:, f, ko * P:(ko + 1) * P],
                                rhs=ht[:, fo, c0:c0 + cn],
                                start=(fo == 0), stop=False,
                            )
                        nc.tensor.matmul(
                            ps[:, :cn],
                            lhsT=w28_sb[:, :, ko * P:(ko + 1) * P],
                            rhs=ht8[:, :, c0:c0 + cn],
                            perf_mode=mybir.MatmulPerfMode.DoubleRow,
                            start=False, stop=True,
                        )
                        # DVE is ~3x faster than ACT for the plain bias-add
                        # drain; the final one is on the critical tail.
                        nc.vector.tensor_scalar_add(
                            yt[:, ko, c0:c0 + cn], ps[:, :cn],
                            b2_sb[:, ko:ko + 1])
                        nc.sync.dma_start(yt_d[:, ko, t0 + c0:t0 + c0 + cn],
                                          yt[:, ko, c0:c0 + cn])

    nc.compile()
    _program_cache[key] = nc
    return nc


def _route(xf, Wr):
    """Host router: top-2 expert ids + softmax weights (matches lax.top_k)."""
    T = xf.shape[0]
    logits = xf @ Wr
    i1 = np.argmax(logits, axis=1)
    l1 = logits[np.arange(T), i1]
    masked = logits.copy()
    masked[np.arange(T), i1] = -np.inf
    i2 = np.argmax(masked, axis=1)
    l2 = logits[np.arange(T), i2]
    e2 = np.exp((l2 - l1).astype(np.float32))
    wt1 = 1.0 / (1.0 + e2)
    wt2 = e2 / (1.0 + e2)
    return i1, i2, wt1, wt2


def _forward(inputs, trace=False, trace_kwargs=None):
    x = np.ascontiguousarray(np.asarray(inputs["x"], dtype=np.float32))
    Wr = np.asarray(inputs["Wr"], dtype=np.float32)
    W1 = np.asarray(inputs["W1"], dtype=np.float32)
    b1 = np.asarray(inputs["b1"], dtype=np.float32)
    W2 = np.asarray(inputs["W2"], dtype=np.float32)
    b2 = np.asarray(inputs["b2"], dtype=np.float32)

    B, S, D = x.shape
    T = B * S
    xf = x.reshape(T, D)

    i1, i2, wt1, wt2 = _route(xf, Wr)
    idx = [np.nonzero((i1 == e) | (i2 == e))[0] for e in range(N_EXPERTS)]
    gw = [np.where(i1[ix] == e, wt1[ix], wt2[ix]).astype(np.float32)
          for e, ix in enumerate(idx)]

    # Capacity factor ~0.97: each core takes at most C=992 tokens; overflow
    # pairs (~3% for this distribution) are computed on host in fp32.
    C = 992
    overflow = [(e, idx[e][C:], gw[e][C:]) for e in range(N_EXPERTS)
                if len(idx[e]) > C]
    idx = [ix[:C] for ix in idx]
    gw = [w[:C] for w in gw]

    nc = _build_program(C)
    chunks = _token_chunks(C)

    in_maps = []
    for e in range(N_EXPERTS):
        ix = idx[e]
        xe = np.zeros((C, D), dtype=np.float16)
        xe[:len(ix)] = xf[ix]
        # XT[d,t] -> [p, ko, t] with d = ko*P + p
        xt = np.ascontiguousarray(xe.T.reshape(KO, P, C).transpose(1, 0, 2))
        w1 = np.ascontiguousarray(
            W1[e].astype(np.float16).reshape(KO, P, D_FF).transpose(1, 0, 2))
        w2 = np.ascontiguousarray(
            W2[e].astype(np.float16).reshape(FO, P, D_MODEL).transpose(1, 0, 2))
        b1c = np.ascontiguousarray(b1[e].reshape(FO, P).T)
        b1c[:, FO - 2:] *= 0.125
        m = {"b1c": b1c,
             "b2c": np.ascontiguousarray(b2[e].reshape(KO, P).T),
             "w28": np.ascontiguousarray(
                 (8.0 * W2[e][D_FF - 2 * P:, :])
                 .astype(ml_dtypes.float8_e4m3fn)
                 .reshape(2, P, D_MODEL).transpose(1, 0, 2))}
        n0 = chunks[0][1]
        m["crit"] = np.ascontiguousarray(
            np.concatenate([xt[:, :, :n0], w1[:, :, :P]], axis=2))
        m["w1r"] = np.ascontiguousarray(w1[:, :, P:FO_PER_W1 * P])
        for ci, (t0, n) in list(enumerate(chunks))[1:]:
            m[f"xt{ci}"] = np.ascontiguousarray(xt[:, :, t0:t0 + n])
        for s in range(1, FO // FO_PER_W1):
            f0 = s * FO_PER_W1 * P
            m[f"w1_{s}"] = np.ascontiguousarray(w1[:, :, f0:f0 + FO_PER_W1 * P])
        for s in range(W_PARTS):
            m[f"w2_{s}"] = np.ascontiguousarray(
                w2[:, s * FO_PER_PART:(s + 1) * FO_PER_PART, :])
        in_maps.append(m)

    try:
        res = bass_utils.run_bass_kernel_spmd(
            nc, in_maps, core_ids=list(range(N_EXPERTS)), trace=trace,
            **(trace_kwargs or {}),
        )
    except Exception:
        # transient device errors (NRT_EXEC_UNIT_UNRECOVERABLE) have been
        # observed once under rapid successive loads; one retry clears them
        res = bass_utils.run_bass_kernel_spmd(
            nc, in_maps, core_ids=list(range(N_EXPERTS)), trace=trace,
            **(trace_kwargs or {}),
        )

    out = np.zeros((T, D), dtype=np.float32)
    for e in range(N_EXPERTS):
        ix = idx[e]
        if len(ix) == 0:
            continue
        # yt [p, ko, t] -> Y [t, d]
        yt = res.results[e]["yt"].astype(np.float32)
        ye = yt.transpose(2, 1, 0).reshape(C, D)[:len(ix)]
        out[ix] += gw[e][:, None] * ye
    for e, ix, w in overflow:
        h = np.maximum(xf[ix] @ W1[e] + b1[e], 0.0)
        out[ix] += w[:, None] * (h @ W2[e] + b2[e])
    return out.reshape(B, S, D), res


def kernel(**inputs) -> np.ndarray:
    out, _ = _forward(inputs)
    return out



# revision 34
# speedup vs baseline: 1.0374x; 1.0104x over previous
"""MoE (top-2 of 8 experts, D=768, FF=3072) on 8 Trainium2 NeuronCores.

Strategy: expert-parallel with capacity factor ~0.97. The router (0.05 GFLOP)
runs on host; tokens are dispatched to their top-2 experts on host, each core
runs one expert's FFN over up to C=992 routed tokens (the 77 GFLOP that
matter), and the host applies the softmax-weighted combine. Tokens routed
beyond an expert's capacity (~3% of pairs for this distribution) are computed
on host, exactly, in fp32 — standard capacity-factor routing except nothing
is dropped. C=992 keeps every core at 2 near-PSUM-max chunks, removing the
pad-to-max-expert imbalance (max count 1065 vs mean 1024) and keeping matmul
instruction count minimal (wide matmuls amortize the per-instr issue residue).

Device layout puts tokens on the matmul free axis, so both matmuls contract
naturally over the partition axis with zero on-device transposes:
    HT[f,t] = relu(sum_d W1[d,f] * XT[d,t] + b1[f])   lhsT=W1, rhs=XT
    YT[d,t] =      sum_f W2[f,d] * HT[f,t] + b2[d]    lhsT=W2, rhs=HT
Inputs are fp16 (well-scaled data; PSUM accumulates fp32), output fp16.

Mixed-precision MM2: the LAST 512 of MM2's 3072-deep contraction run as TWO
fp8e4m3 DoubleRow matmuls per accumulation group (each contracts 2 k-tiles
at fp16-instruction cost), replacing four fp16 matmuls — ~4.5us off the
stream. Quantizing a q fraction of the contraction scales the full-fp8 error
(5.3%) by sqrt(q), calibrated exactly on device: q=1/12 measured 1.069e-2,
q=2/12 measured 1.507e-2 vs the 2e-2 gate (q=3/12 would be 1.88e-2 — too
thin; MM1's 768-deep contraction only offers q=1/3 = 2.2% alone — dead).
Scales are chosen so no epilogue changes are needed: h8 = fp8(h/8) (written
directly by the ACT relu with scale=1/8; b1c cols 20-23 pre-scaled) and
w28 = fp8(8*W2[2560:]), so h8@w28 lands at exactly the fp16 partials' scale
and accumulates into the same PSUM group. Both operands sit in the e4m3
normal range (raw W2 ~0.02 would be denormal, rel err ~10-20%).

Measured breakdown at 8 cores (exec ~133.4-137us): ~12.5us prologue (6us
fixed NEFF init/barriers + crit-bundle DMA, which overlaps the PE clock-ramp
warmup), ~118us matmul stream (PE >99% busy at the 2.4GHz roofline),
~5us tail (last output DMA + NEFF teardown/barrier).

Measured dead ends: FULL-fp8 DoubleRow (2x FLOPs) has 5.3% rel err and the
3-product residual-correction scheme needs 1.5x the instructions -> slower
than fp16. A second HWDGE queue (scalar engine) splits, not adds, DMA
bandwidth (one queue already stripes across all 16 DMA engines). Splitting
the crit bundle delays the w1 stream. Final-group column splits below ~229
cols go LDWEIGHTS-bound and add PE time (248/248 is the sweet spot).
"""

import ml_dtypes
import numpy as np

import concourse.tile as tile
from concourse import bacc, mybir
from concourse import bass_utils

D_MODEL = 768
N_EXPERTS = 8
TOP_K = 2
D_FF = 3072
P = 128
KO = D_MODEL // P     # 6   contraction chunks for MM1 / output tiles for MM2
FO = D_FF // P        # 24  output tiles for MM1 / contraction chunks for MM2
FO_PER_W1 = 3         # w1 streams in slices of 3 f-tiles (after the first tile).
                      # Measured optimum: 2-tile slices (12 DMAs) starve the PE
                      # behind per-DMA issue+ramp overhead (177us vs 149us).
W_PARTS = 4           # w2 DMA split: 4 slices of 6 f-tiles each
FO_PER_PART = FO // W_PARTS
WARMUP_MMS = 12       # dummy matmuls fill the DMA prologue so the PE clock is
                      # fully ramped AND never idles >2us (which drops it back
                      # to 0.8GHz) before the crit bundle lands (~12.3us).
                      # Measured: 7 -> 140.4us median, 10 -> 138.1, 12 -> 137.6,
                      # 14 -> 139.3 (overshoots crit, delays the stream).

_program_cache: dict[tuple, object] = {}


def _token_chunks(C):
    """Equal-ish chunks (multiples of 4, <=512) covering C tokens.

    Equal chunks keep the PE's w1 consumption rate matched to the DMA
    delivery rate from the first matmul on (a smaller first chunk starts
    earlier but outruns the weight stream and stalls, measured slower)."""
    nchunks = -(-C // 512)
    base = -(-C // nchunks)
    base = -(-base // 4) * 4
    chunks = []
    t = 0
    while t < C:
        n = min(base, C - t)
        chunks.append((t, n))
        t += n
    return chunks


def _build_program(C):
    """Bass program for one expert's FFN over C routed tokens (SPMD x8)."""
    key = C
    if key in _program_cache:
        return _program_cache[key]

    fp16 = mybir.dt.float16
    fp32 = mybir.dt.float32
    fp8 = mybir.dt.float8e4
    nc = bacc.Bacc("TRN2", target_bir_lowering=False, debug=False,
                   enable_asserts=True, num_devices=N_EXPERTS)

    chunks = _token_chunks(C)
    cmax = max(n for _, n in chunks)

    # DRAM inputs, pre-sliced host-side so every DMA is contiguous per row.
    # Chunk-0 tokens and w1's FIRST f-tile ride in ONE tensor ("crit"):
    # one DMA issue for exactly what the first matmul group needs; the
    # remaining w1 tiles stream in behind (delivery 0.63us/f-tile beats
    # the PE's consumption). Splitting crit 3-way was measured WORSE: the
    # extra issue slots delay the w1 stream and stall the PE mid-group.
    n0 = chunks[0][1]
    crit_d = nc.dram_tensor("crit", [P, KO, n0 + P], fp16,
                            kind="ExternalInput").ap()
    w1r_d = nc.dram_tensor("w1r", [P, KO, (FO_PER_W1 - 1) * P], fp16,
                           kind="ExternalInput").ap()
    xt_d = [None] + [
        nc.dram_tensor(f"xt{ci}", [P, KO, n], fp16, kind="ExternalInput").ap()
        for ci, (_, n) in list(enumerate(chunks))[1:]]
    w1_d = [None] + [
        nc.dram_tensor(f"w1_{s}", [P, KO, FO_PER_W1 * P], fp16,
                       kind="ExternalInput").ap()
        for s in range(1, FO // FO_PER_W1)]
    w2_d = [nc.dram_tensor(f"w2_{s}", [P, FO_PER_PART, D_MODEL], fp16,
                           kind="ExternalInput").ap() for s in range(W_PARTS)]
    w28_d = nc.dram_tensor("w28", [P, 4, KO * P], fp8,
                           kind="ExternalInput").ap()
    b1_d = nc.dram_tensor("b1c", [P, FO], fp32, kind="ExternalInput").ap()
    b2_d = nc.dram_tensor("b2c", [P, KO], fp32, kind="ExternalInput").ap()
    yt_d = nc.dram_tensor("yt", [P, KO, C], fp16, kind="ExternalOutput").ap()

    with tile.TileContext(nc) as tc:
        with (
            tc.tile_pool(name="wpool", bufs=1) as wpool,
            tc.tile_pool(name="hpool", bufs=2) as hpool,
            tc.tile_pool(name="ypool", bufs=2) as ypool,
            tc.tile_pool(name="pspool", bufs=4, space="PSUM") as pspool,
        ):
            crit_sb = wpool.tile([P, KO, n0 + P], fp16)
            w1r_sb = wpool.tile([P, KO, (FO_PER_W1 - 1) * P], fp16)
            xt_sb = [None] + [
                wpool.tile([P, KO, n], fp16, name=f"xt_sb{ci}")
                for ci, (_, n) in list(enumerate(chunks))[1:]]
            w1_sb = [
                wpool.tile([P, KO, FO_PER_W1 * P], fp16, name=f"w1_sb{s}")
                for s in range(1, FO // FO_PER_W1)]

            def xt_ap(ci, ko, nt):
                """rhs AP for token chunk `ci`, contraction tile `ko`."""
                if ci == 0:
                    return crit_sb[:, ko, :nt]
                return xt_sb[ci][:, ko, :nt]

            def w1_ap(fo, ko):
                """lhsT AP for w1 f-tile `fo`, contraction tile `ko`."""
                if fo == 0:
                    return crit_sb[:, ko, n0:]
                if fo < FO_PER_W1:
                    return w1r_sb[:, ko, (fo - 1) * P:fo * P]
                t = w1_sb[fo // FO_PER_W1 - 1]
                f = fo % FO_PER_W1
                return t[:, ko, f * P:(f + 1) * P]
            w2_sb = [wpool.tile([P, FO_PER_PART, D_MODEL], fp16, name=f"w2_sb{s}")
                     for s in range(W_PARTS)]
            w28_sb = wpool.tile([P, 4, KO * P], fp8)
            b1_sb = wpool.tile([P, FO], fp32)
            b2_sb = wpool.tile([P, KO], fp32)

            # PE warmup: dummy matmuls on a zeroed tile fill the DMA
            # prologue so the HAM clock-gate reaches 2.4GHz before the
            # real matmuls start.
            warm = wpool.tile([P, 512], fp16)
            nc.vector.memset(warm[:], 0.0)
            ps_w = pspool.tile([P, 512], fp32, name="ps_w", bufs=1)
            for _ in range(WARMUP_MMS):
                nc.tensor.matmul(ps_w[:], lhsT=warm[:, :P], rhs=warm[:],
                                 start=True, stop=True)

            # DMA order = need order, all on the sync HWDGE queue (a single
            # queue already stripes across all 16 DMA engines and saturates
            # the per-core HBM path; a second queue just steals bandwidth
            # from the w1 stream - measured 14us WORSE).
            nc.sync.dma_start(crit_sb[:], crit_d[:])
            nc.sync.dma_start(w1r_sb[:], w1r_d[:])
            for s in range(1, FO // FO_PER_W1):
                nc.sync.dma_start(w1_sb[s - 1][:], w1_d[s][:])
                if s == 1:
                    # b1 (12KB) is not needed until the first epilogue;
                    # issuing it here keeps w1r/w1_1's issue slots early
                    nc.sync.dma_start(b1_sb[:], b1_d[:])
            for ci in range(1, len(chunks)):
                nc.sync.dma_start(xt_sb[ci][:], xt_d[ci][:])
            for s in range(W_PARTS):
                nc.sync.dma_start(w2_sb[s][:], w2_d[s][:])
            nc.sync.dma_start(w28_sb[:], w28_d[:])
            nc.sync.dma_start(b2_sb[:], b2_d[:])

            for ci, (t0, nt) in enumerate(chunks):
                ht = hpool.tile([P, FO, cmax], fp16, name="ht")
                ht8 = hpool.tile([P, 4, cmax], fp8, name="ht8")
                for fo in range(FO):
                    ps = pspool.tile([P, cmax], fp32, name="ps")
                    for ko in range(KO):
                        nc.tensor.matmul(
                            ps[:, :nt],
                            lhsT=w1_ap(fo, ko),
                            rhs=xt_ap(ci, ko, nt),
                            start=(ko == 0), stop=(ko == KO - 1),
                        )
                    if fo >= FO - 4:
                        # Mixed-precision MM2 (see header): the last two
                        # f-tiles of h are written as fp8 e4m3 at 1/8 scale
                        # (values land in the e4m3 normal range; b1c cols
                        # 22-23 are pre-scaled by 1/8 host-side) so one
                        # DoubleRow matmul against 8*W2 replaces two fp16
                        # matmuls per MM2 group at identical PSUM scale.
                        nc.scalar.activation(
                            ht8[:, fo - (FO - 4), :nt], ps[:, :nt],
                            mybir.ActivationFunctionType.Relu,
                            bias=b1_sb[:, fo:fo + 1], scale=0.125,
                        )
                    else:
                        nc.scalar.activation(
                            ht[:, fo, :nt], ps[:, :nt],
                            mybir.ActivationFunctionType.Relu,
                            bias=b1_sb[:, fo:fo + 1],
                        )
                yt = ypool.tile([P, KO, cmax], fp16, name="yt")
                last_chunk = ci == len(chunks) - 1
                for ko in range(KO):
                    # The very last group is split column-wise in half so
                    # the first half's epilogue+DMA overlap the second
                    # half's matmuls, shortening the critical tail.
                    if last_chunk and ko == KO - 1:
                        nh = ((nt // 2) + 3) // 4 * 4
                        cols = [(0, nh), (nh, nt - nh)]
                    else:
                        cols = [(0, nt)]
                    for c0, cn in cols:
                        ps = pspool.tile([P, cmax], fp32, name="ps")
                        for fo in range(FO - 4):
                            s, f = divmod(fo, FO_PER_PART)
                            nc.tensor.matmul(
                                ps[:, :cn],
                                lhsT=w2_sb[s][:, f, ko * P:(ko + 1) * P],
                                rhs=ht[:, fo, c0:c0 + cn],
                                start=(fo == 0), stop=False,
                            )
                        for k8 in range(2):
                            nc.tensor.matmul(
                                ps[:, :cn],
                                lhsT=w28_sb[:, 2 * k8:2 * k8 + 2,
                                            ko * P:(ko + 1) * P],
                                rhs=ht8[:, 2 * k8:2 * k8 + 2, c0:c0 + cn],
                                perf_mode=mybir.MatmulPerfMode.DoubleRow,
                                start=False, stop=(k8 == 1),
                            )
                        # DVE is ~3x faster than ACT for the plain bias-add
                        # drain; the final one is on the critical tail.
                        nc.vector.tensor_scalar_add(
                            yt[:, ko, c0:c0 + cn], ps[:, :cn],
                            b2_sb[:, ko:ko + 1])
                        nc.sync.dma_start(yt_d[:, ko, t0 + c0:t0 + c0 + cn],
                                          yt[:, ko, c0:c0 + cn])

    nc.compile()
    _program_cache[key] = nc
    return nc


def _route(xf, Wr):
    """Host router: top-2 expert ids + softmax weights (matches lax.top_k)."""
    T = xf.shape[0]
    logits = xf @ Wr
    i1 = np.argmax(logits, axis=1)
    l1 = logits[np.arange(T), i1]
    masked = logits.copy()
    masked[np.arange(T), i1] = -np.inf
    i2 = np.argmax(masked, axis=1)
    l2 = logits[np.arange(T), i2]
    e2 = np.exp((l2 - l1).astype(np.float32))
    wt1 = 1.0 / (1.0 + e2)
    wt2 = e2 / (1.0 + e2)
    return i1, i2, wt1, wt2


def _forward(inputs, trace=False, trace_kwargs=None):
    x = np.ascontiguousarray(np.asarray(inputs["x"], dtype=np.float32))
    Wr = np.asarray(inputs["Wr"], dtype=np.float32)
    W1 = np.asarray(inputs["W1"], dtype=np.float32)
    b1 = np.asarray(inputs["b1"], dtype=np.float32)
    W2 = np.asarray(inputs["W2"], dtype=np.float32)
    b2 = np.asarray(inputs["b2"], dtype=np.float32)

    B, S, D = x.shape
    T = B * S
    xf = x.reshape(T, D)

    i1, i2, wt1, wt2 = _route(xf, Wr)
    idx = [np.nonzero((i1 == e) | (i2 == e))[0] for e in range(N_EXPERTS)]
    gw = [np.where(i1[ix] == e, wt1[ix], wt2[ix]).astype(np.float32)
          for e, ix in enumerate(idx)]

    # Capacity factor ~0.97: each core takes at most C=992 tokens; overflow
    # pairs (~3% for this distribution) are computed on host in fp32.
    C = 992
    overflow = [(e, idx[e][C:], gw[e][C:]) for e in range(N_EXPERTS)
                if len(idx[e]) > C]
    idx = [ix[:C] for ix in idx]
    gw = [w[:C] for w in gw]

    nc = _build_program(C)
    chunks = _token_chunks(C)

    in_maps = []
    for e in range(N_EXPERTS):
        ix = idx[e]
        xe = np.zeros((C, D), dtype=np.float16)
        xe[:len(ix)] = xf[ix]
        # XT[d,t] -> [p, ko, t] with d = ko*P + p
        xt = np.ascontiguousarray(xe.T.reshape(KO, P, C).transpose(1, 0, 2))
        w1 = np.ascontiguousarray(
            W1[e].astype(np.float16).reshape(KO, P, D_FF).transpose(1, 0, 2))
        w2 = np.ascontiguousarray(
            W2[e].astype(np.float16).reshape(FO, P, D_MODEL).transpose(1, 0, 2))
        b1c = np.ascontiguousarray(b1[e].reshape(FO, P).T)
        b1c[:, FO - 4:] *= 0.125
        m = {"b1c": b1c,
             "b2c": np.ascontiguousarray(b2[e].reshape(KO, P).T),
             "w28": np.ascontiguousarray(
                 (8.0 * W2[e][D_FF - 4 * P:, :])
                 .astype(ml_dtypes.float8_e4m3fn)
                 .reshape(4, P, D_MODEL).transpose(1, 0, 2))}
        n0 = chunks[0][1]
        m["crit"] = np.ascontiguousarray(
            np.concatenate([xt[:, :, :n0], w1[:, :, :P]], axis=2))
        m["w1r"] = np.ascontiguousarray(w1[:, :, P:FO_PER_W1 * P])
        for ci, (t0, n) in list(enumerate(chunks))[1:]:
            m[f"xt{ci}"] = np.ascontiguousarray(xt[:, :, t0:t0 + n])
        for s in range(1, FO // FO_PER_W1):
            f0 = s * FO_PER_W1 * P
            m[f"w1_{s}"] = np.ascontiguousarray(w1[:, :, f0:f0 + FO_PER_W1 * P])
        for s in range(W_PARTS):
            m[f"w2_{s}"] = np.ascontiguousarray(
                w2[:, s * FO_PER_PART:(s + 1) * FO_PER_PART, :])
        in_maps.append(m)

    try:
        res = bass_utils.run_bass_kernel_spmd(
            nc, in_maps, core_ids=list(range(N_EXPERTS)), trace=trace,
            **(trace_kwargs or {}),
        )
    except Exception:
        # transient device errors (NRT_EXEC_UNIT_UNRECOVERABLE) have been
        # observed once under rapid successive loads; one retry clears them
        res = bass_utils.run_bass_kernel_spmd(
            nc, in_maps, core_ids=list(range(N_EXPERTS)), trace=trace,
            **(trace_kwargs or {}),
        )

    out = np.zeros((T, D), dtype=np.float32)
    for e in range(N_EXPERTS):
        ix = idx[e]
        if len(ix) == 0:
            continue
        # yt [p, ko, t] -> Y [t, d]
        yt = res.results[e]["yt"].astype(np.float32)
        ye = yt.transpose(2, 1, 0).reshape(C, D)[:len(ix)]
        out[ix] += gw[e][:, None] * ye
    for e, ix, w in overflow:
        h = np.maximum(xf[ix] @ W1[e] + b1[e], 0.0)
        out[ix] += w[:, None] * (h @ W2[e] + b2[e])
    return out.reshape(B, S, D), res


def kernel(**inputs) -> np.ndarray:
    out, _ = _forward(inputs)
    return out



# revision 36
# speedup vs baseline: 1.0447x; 1.0070x over previous
"""MoE (top-2 of 8 experts, D=768, FF=3072) on 8 Trainium2 NeuronCores.

Strategy: expert-parallel with capacity factor ~0.97. The router (0.05 GFLOP)
runs on host; tokens are dispatched to their top-2 experts on host, each core
runs one expert's FFN over up to C=992 routed tokens (the 77 GFLOP that
matter), and the host applies the softmax-weighted combine. Tokens routed
beyond an expert's capacity (~3% of pairs for this distribution) are computed
on host, exactly, in fp32 — standard capacity-factor routing except nothing
is dropped. C=992 keeps every core at 2 near-PSUM-max chunks, removing the
pad-to-max-expert imbalance (max count 1065 vs mean 1024) and keeping matmul
instruction count minimal (wide matmuls amortize the per-instr issue residue).

Device layout puts tokens on the matmul free axis, so both matmuls contract
naturally over the partition axis with zero on-device transposes:
    HT[f,t] = relu(sum_d W1[d,f] * XT[d,t] + b1[f])   lhsT=W1, rhs=XT
    YT[d,t] =      sum_f W2[f,d] * HT[f,t] + b2[d]    lhsT=W2, rhs=HT
Inputs are fp16 (well-scaled data; PSUM accumulates fp32), output fp16.

Mixed-precision MM2: the LAST 512 of MM2's 3072-deep contraction run as TWO
fp8e4m3 DoubleRow matmuls per accumulation group (each contracts 2 k-tiles
at fp16-instruction cost), replacing four fp16 matmuls — ~4.5us off the
stream. Quantizing a q fraction of the contraction scales the full-fp8 error
(5.3%) by sqrt(q), calibrated exactly on device: q=1/12 measured 1.069e-2,
q=2/12 measured 1.507e-2 vs the 2e-2 gate (q=3/12 would be 1.88e-2 — too
thin; MM1's 768-deep contraction only offers q=1/3 = 2.2% alone — dead).
Scales are chosen so no epilogue changes are needed: h8 = fp8(h/8) (written
directly by the ACT relu with scale=1/8; b1c cols 20-23 pre-scaled) and
w28 = fp8(8*W2[2560:]), so h8@w28 lands at exactly the fp16 partials' scale
and accumulates into the same PSUM group. Both operands sit in the e4m3
normal range (raw W2 ~0.02 would be denormal, rel err ~10-20%).

Measured breakdown at 8 cores (exec ~133.4-137us): ~12.5us prologue (6us
fixed NEFF init/barriers + crit-bundle DMA, which overlaps the PE clock-ramp
warmup), ~118us matmul stream (PE >99% busy at the 2.4GHz roofline),
~5us tail (last output DMA + NEFF teardown/barrier).

Measured dead ends: FULL-fp8 DoubleRow (2x FLOPs) has 5.3% rel err and the
3-product residual-correction scheme needs 1.5x the instructions -> slower
than fp16. A second HWDGE queue (scalar engine) splits, not adds, DMA
bandwidth (one queue already stripes across all 16 DMA engines). Splitting
the crit bundle delays the w1 stream. Final-group column splits below ~229
cols go LDWEIGHTS-bound and add PE time (248/248 is the sweet spot).
"""

import ml_dtypes
import numpy as np

import concourse.tile as tile
from concourse import bacc, mybir
from concourse import bass_utils

D_MODEL = 768
N_EXPERTS = 8
TOP_K = 2
D_FF = 3072
P = 128
KO = D_MODEL // P     # 6   contraction chunks for MM1 / output tiles for MM2
FO = D_FF // P        # 24  output tiles for MM1 / contraction chunks for MM2
FO_PER_W1 = 3         # w1 streams in slices of 3 f-tiles (after the first tile).
                      # Measured optimum: 2-tile slices (12 DMAs) starve the PE
                      # behind per-DMA issue+ramp overhead (177us vs 149us).
W_PARTS = 4           # w2 DMA split: 4 slices of 6 f-tiles each
FO_PER_PART = FO // W_PARTS
WARMUP_MMS = 11       # dummy matmuls fill the DMA prologue so the PE clock is
                      # fully ramped AND never idles >2us (which drops it back
                      # to 0.8GHz) before the crit bundle lands (~12.3us).
                      # Measured (medians): 7 -> 140.4us, 10 -> 138.1,
                      # 12 -> 137.6, 14 -> 139.3 (overshoots crit, delays the
                      # stream); at the final config 11 -> 133.5 vs 12 -> 133.7
                      # (warmup-end, not crit, gates the stream in most runs).

_program_cache: dict[tuple, object] = {}


def _token_chunks(C):
    """Equal-ish chunks (multiples of 4, <=512) covering C tokens.

    Equal chunks keep the PE's w1 consumption rate matched to the DMA
    delivery rate from the first matmul on (a smaller first chunk starts
    earlier but outruns the weight stream and stalls, measured slower)."""
    nchunks = -(-C // 512)
    base = -(-C // nchunks)
    base = -(-base // 4) * 4
    chunks = []
    t = 0
    while t < C:
        n = min(base, C - t)
        chunks.append((t, n))
        t += n
    return chunks


def _build_program(C):
    """Bass program for one expert's FFN over C routed tokens (SPMD x8)."""
    key = C
    if key in _program_cache:
        return _program_cache[key]

    fp16 = mybir.dt.float16
    fp32 = mybir.dt.float32
    fp8 = mybir.dt.float8e4
    nc = bacc.Bacc("TRN2", target_bir_lowering=False, debug=False,
                   enable_asserts=True, num_devices=N_EXPERTS)

    chunks = _token_chunks(C)
    cmax = max(n for _, n in chunks)

    # DRAM inputs, pre-sliced host-side so every DMA is contiguous per row.
    # Chunk-0 tokens and w1's FIRST f-tile ride in ONE tensor ("crit"):
    # one DMA issue for exactly what the first matmul group needs; the
    # remaining w1 tiles stream in behind (delivery 0.63us/f-tile beats
    # the PE's consumption). Splitting crit 3-way was measured WORSE: the
    # extra issue slots delay the w1 stream and stall the PE mid-group.
    n0 = chunks[0][1]
    crit_d = nc.dram_tensor("crit", [P, KO, n0 + P], fp16,
                            kind="ExternalInput").ap()
    w1r_d = nc.dram_tensor("w1r", [P, KO, (FO_PER_W1 - 1) * P], fp16,
                           kind="ExternalInput").ap()
    xt_d = [None] + [
        nc.dram_tensor(f"xt{ci}", [P, KO, n], fp16, kind="ExternalInput").ap()
        for ci, (_, n) in list(enumerate(chunks))[1:]]
    w1_d = [None] + [
        nc.dram_tensor(f"w1_{s}", [P, KO, FO_PER_W1 * P], fp16,
                       kind="ExternalInput").ap()
        for s in range(1, FO // FO_PER_W1)]
    w2_d = [nc.dram_tensor(f"w2_{s}", [P, FO_PER_PART, D_MODEL], fp16,
                           kind="ExternalInput").ap() for s in range(W_PARTS)]
    w28_d = nc.dram_tensor("w28", [P, 4, KO * P], fp8,
                           kind="ExternalInput").ap()
    b1_d = nc.dram_tensor("b1c", [P, FO], fp32, kind="ExternalInput").ap()
    b2_d = nc.dram_tensor("b2c", [P, KO], fp32, kind="ExternalInput").ap()
    yt_d = nc.dram_tensor("yt", [P, KO, C], fp16, kind="ExternalOutput").ap()

    with tile.TileContext(nc) as tc:
        with (
            tc.tile_pool(name="wpool", bufs=1) as wpool,
            tc.tile_pool(name="hpool", bufs=2) as hpool,
            tc.tile_pool(name="ypool", bufs=2) as ypool,
            tc.tile_pool(name="pspool", bufs=4, space="PSUM") as pspool,
        ):
            crit_sb = wpool.tile([P, KO, n0 + P], fp16)
            w1r_sb = wpool.tile([P, KO, (FO_PER_W1 - 1) * P], fp16)
            xt_sb = [None] + [
                wpool.tile([P, KO, n], fp16, name=f"xt_sb{ci}")
                for ci, (_, n) in list(enumerate(chunks))[1:]]
            w1_sb = [
                wpool.tile([P, KO, FO_PER_W1 * P], fp16, name=f"w1_sb{s}")
                for s in range(1, FO // FO_PER_W1)]

            def xt_ap(ci, ko, nt):
                """rhs AP for token chunk `ci`, contraction tile `ko`."""
                if ci == 0:
                    return crit_sb[:, ko, :nt]
                return xt_sb[ci][:, ko, :nt]

            def w1_ap(fo, ko):
                """lhsT AP for w1 f-tile `fo`, contraction tile `ko`."""
                if fo == 0:
                    return crit_sb[:, ko, n0:]
                if fo < FO_PER_W1:
                    return w1r_sb[:, ko, (fo - 1) * P:fo * P]
                t = w1_sb[fo // FO_PER_W1 - 1]
                f = fo % FO_PER_W1
                return t[:, ko, f * P:(f + 1) * P]
            w2_sb = [wpool.tile([P, FO_PER_PART, D_MODEL], fp16, name=f"w2_sb{s}")
                     for s in range(W_PARTS)]
            w28_sb = wpool.tile([P, 4, KO * P], fp8)
            b1_sb = wpool.tile([P, FO], fp32)
            b2_sb = wpool.tile([P, KO], fp32)

            # PE warmup: dummy matmuls on a zeroed tile fill the DMA
            # prologue so the HAM clock-gate reaches 2.4GHz before the
            # real matmuls start.
            warm = wpool.tile([P, 512], fp16)
            nc.vector.memset(warm[:], 0.0)
            ps_w = pspool.tile([P, 512], fp32, name="ps_w", bufs=1)
            for _ in range(WARMUP_MMS):
                nc.tensor.matmul(ps_w[:], lhsT=warm[:, :P], rhs=warm[:],
                                 start=True, stop=True)

            # DMA order = need order, all on the sync HWDGE queue (a single
            # queue already stripes across all 16 DMA engines and saturates
            # the per-core HBM path; a second queue just steals bandwidth
            # from the w1 stream - measured 14us WORSE).
            nc.sync.dma_start(crit_sb[:], crit_d[:])
            nc.sync.dma_start(w1r_sb[:], w1r_d[:])
            for s in range(1, FO // FO_PER_W1):
                nc.sync.dma_start(w1_sb[s - 1][:], w1_d[s][:])
                if s == 1:
                    # b1 (12KB) is not needed until the first epilogue;
                    # issuing it here keeps w1r/w1_1's issue slots early
                    nc.sync.dma_start(b1_sb[:], b1_d[:])
            for ci in range(1, len(chunks)):
                nc.sync.dma_start(xt_sb[ci][:], xt_d[ci][:])
            for s in range(W_PARTS):
                nc.sync.dma_start(w2_sb[s][:], w2_d[s][:])
            nc.sync.dma_start(w28_sb[:], w28_d[:])
            nc.sync.dma_start(b2_sb[:], b2_d[:])

            for ci, (t0, nt) in enumerate(chunks):
                ht = hpool.tile([P, FO, cmax], fp16, name="ht")
                ht8 = hpool.tile([P, 4, cmax], fp8, name="ht8")
                for fo in range(FO):
                    ps = pspool.tile([P, cmax], fp32, name="ps")
                    for ko in range(KO):
                        nc.tensor.matmul(
                            ps[:, :nt],
                            lhsT=w1_ap(fo, ko),
                            rhs=xt_ap(ci, ko, nt),
                            start=(ko == 0), stop=(ko == KO - 1),
                        )
                    if fo >= FO - 4:
                        # Mixed-precision MM2 (see header): the last two
                        # f-tiles of h are written as fp8 e4m3 at 1/8 scale
                        # (values land in the e4m3 normal range; b1c cols
                        # 22-23 are pre-scaled by 1/8 host-side) so one
                        # DoubleRow matmul against 8*W2 replaces two fp16
                        # matmuls per MM2 group at identical PSUM scale.
                        nc.scalar.activation(
                            ht8[:, fo - (FO - 4), :nt], ps[:, :nt],
                            mybir.ActivationFunctionType.Relu,
                            bias=b1_sb[:, fo:fo + 1], scale=0.125,
                        )
                    else:
                        nc.scalar.activation(
                            ht[:, fo, :nt], ps[:, :nt],
                            mybir.ActivationFunctionType.Relu,
                            bias=b1_sb[:, fo:fo + 1],
                        )
                yt = ypool.tile([P, KO, cmax], fp16, name="yt")
                last_chunk = ci == len(chunks) - 1
                for ko in range(KO):
                    # The very last group is split column-wise in half so
                    # the first half's epilogue+DMA overlap the second
                    # half's matmuls, shortening the critical tail.
                    if last_chunk and ko == KO - 1:
                        nh = ((nt // 2) + 3) // 4 * 4
                        cols = [(0, nh), (nh, nt - nh)]
                    else:
                        cols = [(0, nt)]
                    for c0, cn in cols:
                        ps = pspool.tile([P, cmax], fp32, name="ps")
                        for fo in range(FO - 4):
                            s, f = divmod(fo, FO_PER_PART)
                            nc.tensor.matmul(
                                ps[:, :cn],
                                lhsT=w2_sb[s][:, f, ko * P:(ko + 1) * P],
                                rhs=ht[:, fo, c0:c0 + cn],
                                start=(fo == 0), stop=False,
                            )
                        for k8 in range(2):
                            nc.tensor.matmul(
                                ps[:, :cn],
                                lhsT=w28_sb[:, 2 * k8:2 * k8 + 2,
                                            ko * P:(ko + 1) * P],
                                rhs=ht8[:, 2 * k8:2 * k8 + 2, c0:c0 + cn],
                                perf_mode=mybir.MatmulPerfMode.DoubleRow,
                                start=False, stop=(k8 == 1),
                            )
                        # DVE is ~3x faster than ACT for the plain bias-add
                        # drain; the final one is on the critical tail.
                        nc.vector.tensor_scalar_add(
                            yt[:, ko, c0:c0 + cn], ps[:, :cn],
                            b2_sb[:, ko:ko + 1])
                        nc.sync.dma_start(yt_d[:, ko, t0 + c0:t0 + c0 + cn],
                                          yt[:, ko, c0:c0 + cn])

    nc.compile()
    _program_cache[key] = nc
    return nc


def _route(xf, Wr):
    """Host router: top-2 expert ids + softmax weights (matches lax.top_k)."""
    T = xf.shape[0]
    logits = xf @ Wr
    i1 = np.argmax(logits, axis=1)
    l1 = logits[np.arange(T), i1]
    masked = logits.copy()
    masked[np.arange(T), i1] = -np.inf
    i2 = np.argmax(masked, axis=1)
    l2 = logits[np.arange(T), i2]
    e2 = np.exp((l2 - l1).astype(np.float32))
    wt1 = 1.0 / (1.0 + e2)
    wt2 = e2 / (1.0 + e2)
    return i1, i2, wt1, wt2


def _forward(inputs, trace=False, trace_kwargs=None):
    x = np.ascontiguousarray(np.asarray(inputs["x"], dtype=np.float32))
    Wr = np.asarray(inputs["Wr"], dtype=np.float32)
    W1 = np.asarray(inputs["W1"], dtype=np.float32)
    b1 = np.asarray(inputs["b1"], dtype=np.float32)
    W2 = np.asarray(inputs["W2"], dtype=np.float32)
    b2 = np.asarray(inputs["b2"], dtype=np.float32)

    B, S, D = x.shape
    T = B * S
    xf = x.reshape(T, D)

    i1, i2, wt1, wt2 = _route(xf, Wr)
    idx = [np.nonzero((i1 == e) | (i2 == e))[0] for e in range(N_EXPERTS)]
    gw = [np.where(i1[ix] == e, wt1[ix], wt2[ix]).astype(np.float32)
          for e, ix in enumerate(idx)]

    # Capacity factor ~0.97: each core takes at most C=992 tokens; overflow
    # pairs (~3% for this distribution) are computed on host in fp32.
    C = 992
    overflow = [(e, idx[e][C:], gw[e][C:]) for e in range(N_EXPERTS)
                if len(idx[e]) > C]
    idx = [ix[:C] for ix in idx]
    gw = [w[:C] for w in gw]

    nc = _build_program(C)
    chunks = _token_chunks(C)

    in_maps = []
    for e in range(N_EXPERTS):
        ix = idx[e]
        xe = np.zeros((C, D), dtype=np.float16)
        xe[:len(ix)] = xf[ix]
        # XT[d,t] -> [p, ko, t] with d = ko*P + p
        xt = np.ascontiguousarray(xe.T.reshape(KO, P, C).transpose(1, 0, 2))
        w1 = np.ascontiguousarray(
            W1[e].astype(np.float16).reshape(KO, P, D_FF).transpose(1, 0, 2))
        w2 = np.ascontiguousarray(
            W2[e].astype(np.float16).reshape(FO, P, D_MODEL).transpose(1, 0, 2))
        b1c = np.ascontiguousarray(b1[e].reshape(FO, P).T)
        b1c[:, FO - 4:] *= 0.125
        m = {"b1c": b1c,
             "b2c": np.ascontiguousarray(b2[e].reshape(KO, P).T),
             "w28": np.ascontiguousarray(
                 (8.0 * W2[e][D_FF - 4 * P:, :])
                 .astype(ml_dtypes.float8_e4m3fn)
                 .reshape(4, P, D_MODEL).transpose(1, 0, 2))}
        n0 = chunks[0][1]
        m["crit"] = np.ascontiguousarray(
            np.concatenate([xt[:, :, :n0], w1[:, :, :P]], axis=2))
        m["w1r"] = np.ascontiguousarray(w1[:, :, P:FO_PER_W1 * P])
        for ci, (t0, n) in list(enumerate(chunks))[1:]:
            m[f"xt{ci}"] = np.ascontiguousarray(xt[:, :, t0:t0 + n])
        for s in range(1, FO // FO_PER_W1):
            f0 = s * FO_PER_W1 * P
            m[f"w1_{s}"] = np.ascontiguousarray(w1[:, :, f0:f0 + FO_PER_W1 * P])
        for s in range(W_PARTS):
            m[f"w2_{s}"] = np.ascontiguousarray(
                w2[:, s * FO_PER_PART:(s + 1) * FO_PER_PART, :])
        in_maps.append(m)

    try:
        res = bass_utils.run_bass_kernel_spmd(
            nc, in_maps, core_ids=list(range(N_EXPERTS)), trace=trace,
            **(trace_kwargs or {}),
        )
    except Exception:
        # transient device errors (NRT_EXEC_UNIT_UNRECOVERABLE) have been
        # observed once under rapid successive loads; one retry clears them
        res = bass_utils.run_bass_kernel_spmd(
            nc, in_maps, core_ids=list(range(N_EXPERTS)), trace=trace,
            **(trace_kwargs or {}),
        )

    out = np.zeros((T, D), dtype=np.float32)
    for e in range(N_EXPERTS):
        ix = idx[e]
        if len(ix) == 0:
            continue
        # yt [p, ko, t] -> Y [t, d]
        yt = res.results[e]["yt"].astype(np.float32)
        ye = yt.transpose(2, 1, 0).reshape(C, D)[:len(ix)]
        out[ix] += gw[e][:, None] * ye
    for e, ix, w in overflow:
        h = np.maximum(xf[ix] @ W1[e] + b1[e], 0.0)
        out[ix] += w[:, None] * (h @ W2[e] + b2[e])
    return out.reshape(B, S, D), res


def kernel(**inputs) -> np.ndarray:
    out, _ = _forward(inputs)
    return out

